# revision 35
# baseline (speedup 1.0000x reference)
"""GCN/GAT 4-layer GNN on 8 Trainium2 NeuronCores.

Strategy (vertex-cut data parallelism, v2):
  - dst-nodes sharded 8 ways (6250/core); each core owns all edges into its
    shard. Edges sorted by (dst_block, src_half), padded per (block, half)
    to a cross-core-uniform chunk count K[b,h]; pad indices are -1 so the
    SWDGE gather ucode trims them (no descriptors generated for padding).
  - Dense per-node matmuls on the owner core; per-layer tables (node
    features + src attention logits es + dst logits ed) AllGathered so any
    core can gather any src row.  Feature columns stored head-innermost
    (j*8+h) so per-head broadcasts vectorize on DVE.
  - Message passing per dst-block: dma_gather fetches the block's edge rows
    (one 128-edge chunk per gather element group); host-precomputed one-hot
    matrices (identical for all 4 layers) map edges to local dst slots; the
    segmented softmax/sum becomes PSUM-accumulated matmuls.  The GAT softmax
    denominator is streamed as 8 extra rhs columns of the same matmul (L1).
  - ed[dst] per edge comes from a transposed one-hot matmul against the
    block's own table rows (read back from the local bounce buffer) - no
    second dst-side gather.  exp(leaky_relu(x)) = Exp(Lrelu(x)) runs on the
    otherwise-idle Activation engine.
  - PyG's implicit self-loops never enter the edge stream; their GAT/GCN
    contributions are added per block from the readback rows.
"""

import sys, os
for _p in ("/opt/trn_rl_repo", "/root/.axon_site/_ro/trn_rl_repo"):
    if os.path.isdir(_p) and _p not in sys.path:
        sys.path.insert(0, _p)

import numpy as np
import ml_dtypes

import concourse.bass as bass
import concourse.bacc as bacc
import concourse.mybir as mybir
import concourse.tile as tile
from concourse.bass_utils import run_bass_kernel_spmd

F32 = mybir.dt.float32
BF16 = mybir.dt.bfloat16
I16 = mybir.dt.int16
BF = ml_dtypes.bfloat16
AF = mybir.ActivationFunctionType
OP = mybir.AluOpType

P = 8
NEG_SLOPE = 0.2

# per-layer table geometry:
#   L1: row=[h1(256,jh)|es1(8)|ed1(8)|pad] 384   GAT C=256 ch=32
#   L2: row=[dis*h2(128)] 128                    GCN C=128
#   L3: row=[h3(512,jh)|es3(8)|ed3(8)|pad] 640   GAT C=512 ch=64
#   L4: row=[dis*x3(64)|pad] 128                 GCN C=64
LAYERS = [
    dict(row=384, C=256, es=256, ed=264, ch=32, gat=True),
    dict(row=128, C=128, es=None, ed=None, ch=None, gat=False),
    dict(row=640, C=512, es=512, ed=520, ch=64, gat=True),
    dict(row=128, C=64, es=None, ed=None, ch=None, gat=False),
]


def _wrap_idx(idx, cap):
    """Wrap idx (len<=cap, cap%128==0) into [128, cap//16] i16; slot i at
    (i%16, i//16), replicated 8x down partitions; unused slots = -1 (the
    gather ucode trims trailing negatives - no descriptors for padding)."""
    cols = cap // 16
    arr = np.zeros((16, cols), np.int64)
    for off in range(16):
        sub = idx[off::16]
        arr[off, : len(sub)] = sub
    return np.tile(arr, (8, 1)).astype(np.int16)


def preprocess(edge_index, N, trim=True):
    NSH = N // P
    NB = (NSH + 127) // 128
    BA = min(NB - 1, 31)             # blocks in table part A (int16 max)
    RA = min(NSH, BA * 128)          # local rows in part A
    RB = NSH - RA

    src = np.asarray(edge_index[0]).astype(np.int64)
    dst = np.asarray(edge_index[1]).astype(np.int64)
    # degrees include the implicit self loop
    deg = (np.bincount(dst, minlength=N) + 1).astype(np.float64)
    dis = (1.0 / np.sqrt(deg)).astype(np.float32)

    core = dst // NSH
    blk = (dst % NSH) // 128
    # table part of the SOURCE row (A: local row < RA, B: rest)
    s_core = src // NSH
    s_loc = src % NSH
    hi = (s_loc >= RA).astype(np.int64)
    tidx = np.where(hi, s_core * RB + (s_loc - RA), s_core * RA + s_loc)

    counts = np.zeros((P, NB, 2), np.int64)
    np.add.at(counts, (core, blk, hi), 1)
    K = np.maximum(1, -(-counts // 128)).max(axis=0)     # [NB, 2] shared chunks
    # static num_idxs per (block, part) call: the cross-core max real count
    # (not the 128-rounded capacity) - the gather ucode position-masks lanes
    # beyond it, skipping ~5% of descriptor generation with no extra deps
    mx = np.maximum(counts.max(axis=0), 1) if trim else K * 128
    minn = counts.min(axis=0)                            # [NB, 2] min real edges
    TOTCH = int(K.sum())
    # chunk offset of (b, h) in global order (b: partA, partB)
    coff = np.zeros((NB, 2), np.int64)
    run = 0
    for b in range(NB):
        for h in range(2):
            coff[b, h] = run
            run += int(K[b, h])

    order = np.lexsort((hi, blk, core))
    so_tidx, so_core, so_dst, so_blk, so_hi = (
        a[order] for a in (tidx, core, dst, blk, hi))

    percore = []
    for c in range(P):
        m = so_core == c
        csrc, cdst, cblk, chi = so_tidx[m], so_dst[m], so_blk[m], so_hi[m]
        sidx = np.zeros((128, TOTCH * 8), np.int16)
        dloc = np.full((TOTCH * 128,), -1, np.int64)     # slot -> dst_local
        ptr = 0
        for b in range(NB):
            for h in range(2):
                n_e = int(counts[c, b, h])
                cap = int(K[b, h]) * 128
                e_src = csrc[ptr: ptr + n_e]
                e_dst = cdst[ptr: ptr + n_e]
                ptr += n_e
                o = int(coff[b, h])
                sidx[:, o * 8: o * 8 + cap // 16] = _wrap_idx(e_src, cap)
                dloc[o * 128: o * 128 + n_e] = e_dst - c * NSH - b * 128
        assert ptr == int(m.sum())
        dl = dloc.reshape(TOTCH, 128)                    # [chunk, lane]
        d_ar = np.arange(128)
        oh = (dl[:, :, None] == d_ar[None, None, :])     # [ch, lane, d]
        oh_p = np.ascontiguousarray(
            oh.transpose(1, 0, 2)).astype(BF)            # [lane, ch, d]
        ohT_p = np.ascontiguousarray(
            oh.transpose(2, 0, 1)).astype(BF)            # [d, ch, lane]
        dis_c = np.zeros((128, NB), np.float32)
        dv = dis[c * NSH: (c + 1) * NSH]
        for b in range(NB):
            r = min(128, NSH - b * 128)
            dis_c[0:r, b] = dv[b * 128: b * 128 + r]
        ncnt = np.tile(counts[c].reshape(1, NB * 2), (128, 1)).astype(np.int32)
        percore.append(dict(sidx=sidx, oh=oh_p, ohT=ohT_p, dis=dis_c, ncnt=ncnt))
    sched = dict(N=N, NSH=NSH, NB=NB, BA=BA, RA=RA, RB=RB, K=K, coff=coff,
                 TOTCH=TOTCH, minn=minn, mx=mx)
    return sched, percore


def _permjh(W, heads, ch):
    """Reorder last-dim cols from (h*ch+j) to (j*heads+h)."""
    s = W.shape[:-1]
    return np.ascontiguousarray(
        W.reshape(*s, heads, ch).swapaxes(-1, -2).reshape(*s, heads * ch))


def prep_weights(w):
    def b16(a):
        return np.asarray(a, np.float32).astype(BF)

    W1 = np.asarray(w["g1_W"], np.float32)
    ws1s = np.einsum("khj,hj->kh", W1.reshape(1024, 8, 32), np.asarray(w["g1_as"], np.float32))
    ws1d = np.einsum("khj,hj->kh", W1.reshape(1024, 8, 32), np.asarray(w["g1_ad"], np.float32))
    m1 = np.asarray(w["m1_W"], np.float32)
    w1cat = np.concatenate([_permjh(W1, 8, 32), _permjh(m1, 8, 32), ws1s, ws1d], axis=1)
    w1t = b16(w1cat).reshape(8, 128, 528).transpose(1, 0, 2).copy()     # [128, 8, 528]

    # rows of L2 weights permuted to match x1's (j,h) layout
    w2cat = np.concatenate([np.asarray(w["g2_W"], np.float32),
                            np.asarray(w["m2_W"], np.float32)], axis=1)
    w2cat = np.ascontiguousarray(w2cat.reshape(8, 32, 256).transpose(1, 0, 2).reshape(256, 256))
    w2t = b16(w2cat).reshape(2, 128, 256).transpose(1, 0, 2).copy()     # [128, 2, 256]

    W3 = np.asarray(w["g3_W"], np.float32)
    ws3s = np.einsum("khj,hj->kh", W3.reshape(128, 8, 64), np.asarray(w["g3_as"], np.float32))
    ws3d = np.einsum("khj,hj->kh", W3.reshape(128, 8, 64), np.asarray(w["g3_ad"], np.float32))
    w3t = b16(np.concatenate([_permjh(W3, 8, 64), np.asarray(w["m3_W"], np.float32),
                              ws3s, ws3d], axis=1))                     # [128, 592]

    w4t = b16(np.concatenate([np.asarray(w["g4_W"], np.float32),
                              np.asarray(w["m4_W"], np.float32)], axis=1))  # [64, 4]

    def rep(v):
        return np.tile(np.asarray(v, np.float32)[None, :], (128, 1)).copy()

    b1 = _permjh((np.asarray(w["g1_b"]) + np.asarray(w["m1_b"]))[None, :], 8, 32)[0]
    return dict(
        w1=w1t, w2=w2t, w3=w3t, w4=w4t,
        bias1=rep(b1),
        bias2=rep(np.asarray(w["g2_b"]) + np.asarray(w["m2_b"])),
        bias3=rep(np.asarray(w["g3_b"]) + np.asarray(w["m3_b"])),
        bias4=rep(np.asarray(w["g4_b"]) + np.asarray(w["m4_b"])),
        ident=np.eye(128, dtype=np.float32).astype(BF),
    )


def bcast(ap, count):
    """Append a 0-stride broadcast dim of `count` to an AP."""
    return bass.AP(ap.tensor, ap.offset, list(ap.ap) + [[0, count]])


def jh_bcast(ap, ch):
    """[128, k, 8] -> broadcast as [128, k, ch, 8] (j outer, h inner)."""
    a = list(ap.ap)
    return bass.AP(ap.tensor, ap.offset, a[:-1] + [[0, ch], a[-1]])


def build_nc(sched):
    N, NSH, NB, K, coff, TOTCH = (sched["N"], sched["NSH"], sched["NB"],
                                  sched["K"], sched["coff"], sched["TOTCH"])
    BA, RA, RB, minn = sched["BA"], sched["RA"], sched["RB"], sched["minn"]
    mx = sched["mx"]
    nc = bacc.Bacc("TRN2", target_bir_lowering=False, debug=False, num_devices=P)

    # ---- I/O ----
    xT_in = nc.dram_tensor("xT", [NB, 128, 1024], BF16, kind="ExternalInput")
    w1_in = nc.dram_tensor("w1", [128, 8, 528], BF16, kind="ExternalInput")
    w2_in = nc.dram_tensor("w2", [128, 2, 256], BF16, kind="ExternalInput")
    w3_in = nc.dram_tensor("w3", [128, 592], BF16, kind="ExternalInput")
    w4_in = nc.dram_tensor("w4", [64, 4], BF16, kind="ExternalInput")
    b1_in = nc.dram_tensor("bias1", [128, 256], F32, kind="ExternalInput")
    b2_in = nc.dram_tensor("bias2", [128, 128], F32, kind="ExternalInput")
    b3_in = nc.dram_tensor("bias3", [128, 64], F32, kind="ExternalInput")
    b4_in = nc.dram_tensor("bias4", [128, 2], F32, kind="ExternalInput")
    id_in = nc.dram_tensor("ident", [128, 128], BF16, kind="ExternalInput")
    dis_in = nc.dram_tensor("dis", [128, NB], F32, kind="ExternalInput")
    sidx_in = nc.dram_tensor("sidx", [128, TOTCH * 8], I16, kind="ExternalInput")
    oh_in = nc.dram_tensor("oh", [128, TOTCH, 128], BF16, kind="ExternalInput")
    ohT_in = nc.dram_tensor("ohT", [128, TOTCH, 128], BF16, kind="ExternalInput")
    out_dram = nc.dram_tensor("out", [NSH, 2], F32, kind="ExternalOutput")

    TbA = [nc.dram_tensor(f"T{l}bA", [RA, LAYERS[l - 1]["row"]], BF16) for l in range(1, 5)]
    TbB = [nc.dram_tensor(f"T{l}bB", [RB, LAYERS[l - 1]["row"]], BF16) for l in range(1, 5)]
    TgA = [nc.dram_tensor(f"T{l}A", [P * RA, LAYERS[l - 1]["row"]], BF16, addr_space="Shared")
           for l in range(1, 5)]
    TgB = [nc.dram_tensor(f"T{l}B", [P * RB, LAYERS[l - 1]["row"]], BF16, addr_space="Shared")
           for l in range(1, 5)]

    blk_rows = [min(128, NSH - b * 128) for b in range(NB)]

    def tb_rows(l, b):
        """Local bounce-table slice for block b of layer-l table."""
        r = blk_rows[b]
        if b < BA:
            return TbA[l - 1][b * 128: b * 128 + r, :]
        return TbB[l - 1][(b - BA) * 128: (b - BA) * 128 + r, :]

    with tile.TileContext(nc) as tc:
        with (
            tc.tile_pool(name="consts", bufs=1) as cpool,
            tc.tile_pool(name="resident", bufs=1) as rpool,
            tc.tile_pool(name="xload", bufs=2) as xpool,
            tc.tile_pool(name="gath", bufs=3) as gpool,
            tc.tile_pool(name="ohp", bufs=1) as ohpool,
            tc.tile_pool(name="scp", bufs=1) as scpool,
            tc.tile_pool(name="hbp", bufs=2) as hbpool,
            tc.tile_pool(name="work", bufs=2) as wpool,
            tc.tile_pool(name="asm", bufs=2) as apool,
            tc.tile_pool(name="psA", bufs=2, space="PSUM") as psA,
            tc.tile_pool(name="psS", bufs=2, space="PSUM") as psS,
            tc.tile_pool(name="psE", bufs=2, space="PSUM") as psE,
        ):
            # ---------- constants ----------
            w1_t = cpool.tile([128, 8, 528], BF16); nc.sync.dma_start(w1_t[:], w1_in[:])
            w2_t = cpool.tile([128, 2, 256], BF16); nc.sync.dma_start(w2_t[:], w2_in[:])
            w3_t = cpool.tile([128, 592], BF16); nc.sync.dma_start(w3_t[:], w3_in[:])
            w4_t = cpool.tile([64, 4], BF16); nc.sync.dma_start(w4_t[:], w4_in[:])
            b1_t = cpool.tile([128, 256], F32); nc.sync.dma_start(b1_t[:], b1_in[:])
            b2_t = cpool.tile([128, 128], F32); nc.sync.dma_start(b2_t[:], b2_in[:])
            b3_t = cpool.tile([128, 64], F32); nc.sync.dma_start(b3_t[:], b3_in[:])
            b4_t = cpool.tile([128, 2], F32); nc.sync.dma_start(b4_t[:], b4_in[:])
            id_t = cpool.tile([128, 128], BF16); nc.sync.dma_start(id_t[:], id_in[:])
            dis_t = cpool.tile([128, NB], F32); nc.sync.dma_start(dis_t[:], dis_in[:])
            epscol = cpool.tile([128, 1], F32); nc.vector.memset(epscol[:], 1e-5)
            sidx_t = cpool.tile([128, TOTCH * 8], I16); nc.sync.dma_start(sidx_t[:], sidx_in[:])

            mlp_sb = [rpool.tile([128, NB, c], BF16, name=f"mlp{i}_sb", tag=f"mlp{i}_sb")
                      for i, c in enumerate((256, 128, 64))]
            x3_sb = rpool.tile([128, NB, 64], BF16)

            # one-time primes: lanes past each call's max count stay unwritten;
            # keep them finite (zero) so 0-weighted one-hot columns stay 0*x
            KTMX = int(K.sum(axis=1).max())
            for _ in range(2):
                gprime = gpool.tile([128, KTMX, 640], BF16, tag="G")
                nc.vector.memset(gprime[:], 0.0)

            # prime rotating gather/readback buffers so untouched (trimmed)
            # regions hold finite values rather than uninitialized SBUF

            def allgather(l, part):
                tb, tg = (TbA, TgA) if part == 0 else (TbB, TgB)
                nc.gpsimd.collective_compute(
                    "AllGather", OP.bypass, replica_groups=[list(range(P))],
                    ins=[tb[l - 1].ap().opt()], outs=[tg[l - 1].ap().opt()])

            # ---------- dense phase 1: h1|mlp1|es1|ed1 from x ----------
            for b in range(NB):
                xt = xpool.tile([128, 1024], BF16)
                nc.sync.dma_start(xt[:], xT_in[b])
                pd = psA.tile([128, 512], F32, tag="pd")
                pe = psS.tile([128, 16], F32, tag="psm")
                for k in range(8):
                    nc.tensor.matmul(pd[:], xt[:, k * 128:(k + 1) * 128],
                                     w1_t[:, k, 0:512], start=(k == 0), stop=(k == 7))
                for k in range(8):
                    nc.tensor.matmul(pe[:], xt[:, k * 128:(k + 1) * 128],
                                     w1_t[:, k, 512:528], start=(k == 0), stop=(k == 7))
                as1 = apool.tile([128, 384], BF16, tag="as1")
                nc.vector.memset(as1[:, 272:384], 0.0)
                nc.scalar.copy(as1[:, 0:256], pd[:, 0:256])
                nc.vector.tensor_copy(as1[:, 256:272], pe[:, 0:16])
                nc.sync.dma_start(tb_rows(1, b), as1[0:blk_rows[b], :])
                nc.vector.tensor_tensor(mlp_sb[0][:, b, :], pd[:, 256:512], b1_t[:], OP.add)
                if b == BA - 1:
                    allgather(1, 0)
            allgather(1, 1)

            # ---------- propagation layer ----------
            def prop(l, epilogue):
                cfg = LAYERS[l - 1]
                row, C, gat = cfg["row"], cfg["C"], cfg["gat"]
                ch = cfg["ch"]
                for b in range(NB):
                    klo, khi = int(K[b, 0]), int(K[b, 1])
                    kt = klo + khi
                    o = int(coff[b, 0])
                    G = gpool.tile([128, kt, row], BF16, tag="G")
                    nc.gpsimd.dma_gather(
                        out_ap=G[:, 0:klo, :], in_ap=TgA[l - 1][:],
                        idxs_ap=sidx_t[:, o * 8: o * 8 + (int(mx[b, 0]) + 15) // 16],
                        num_idxs=int(mx[b, 0]), num_idxs_reg=int(mx[b, 0]),
                        elem_size=row, single_packet=False)
                    nc.gpsimd.dma_gather(
                        out_ap=G[:, klo:kt, :], in_ap=TgB[l - 1][:],
                        idxs_ap=sidx_t[:, (o + klo) * 8: (o + klo) * 8 + (int(mx[b, 1]) + 15) // 16],
                        num_idxs=int(mx[b, 1]), num_idxs_reg=int(mx[b, 1]),
                        elem_size=row, single_packet=False)
                    # own-shard rows: h_b (self loops) + ed_b (attention rhs)
                    hb = hbpool.tile([128, row], BF16, tag="hb")
                    if blk_rows[b] < 128:
                        nc.vector.memset(hb[:], 0.0)
                    nc.sync.dma_start(hb[0:blk_rows[b], :], tb_rows(l, b))
                    pagg = psA.tile([128, C + 8 if C == 256 else C],
                                    F32, tag="pagg")
                    if True:
                        oh_t = ohpool.tile([128, kt, 128], BF16, tag="oh")
                        nc.sync.dma_start(oh_t[:], oh_in[:, o: o + kt, :])
                        ohT_t = ohpool.tile([128, kt, 128], BF16, tag="ohT")
                        nc.sync.dma_start(ohT_t[:], ohT_in[:, o: o + kt, :])
                        # ed[dst_e] via transposed one-hot @ local ed rows
                        pE = psE.tile([128, kt, 8], F32, tag="pse")
                        for c in range(kt):
                            nc.tensor.matmul(pE[:, c, :], ohT_t[:, c, :],
                                             hb[:, cfg["ed"]: cfg["ed"] + 8],
                                             start=True, stop=True,
                                             skip_group_check=True)
                        tsum = wpool.tile([128, kt, 8], BF16, tag="tsum")
                        nc.vector.tensor_tensor(
                            tsum[:], pE[:],
                            G[:, :, cfg["es"]: cfg["es"] + 8], OP.add)
                        # exp(leaky_relu(x)) == max(exp(x), exp(0.2x))
                        e1 = wpool.tile([128, kt, 8], BF16, tag="e1")
                        nc.scalar.activation(e1[:], tsum[:], AF.Exp)
                        e2 = wpool.tile([128, kt, 8], BF16, tag="e2")
                        nc.scalar.activation(e2[:], tsum[:], AF.Exp, scale=NEG_SLOPE)
                        # sc = [G_h * ex | ex]; ex lands in the tail columns
                        sc = scpool.tile([128, kt, C + 8], BF16, tag="sc")
                        nc.vector.tensor_tensor(sc[:, :, C: C + 8], e1[:], e2[:], OP.max)
                        nc.vector.tensor_tensor(
                            sc[:, :, 0:C].rearrange("p k (j h) -> p k j h", h=8),
                            G[:, :, 0:C].rearrange("p k (j h) -> p k j h", h=8),
                            jh_bcast(sc[:, :, C: C + 8], ch), OP.mult)
                        if C == 256:   # L1: denominator merged into agg matmul
                            for c in range(kt):
                                nc.tensor.matmul(pagg[:], oh_t[:, c, :], sc[:, c, :],
                                                 start=(c == 0), stop=(c == kt - 1),
                                                 skip_group_check=True)
                            ptail = pagg[:, C: C + 8]
                        else:          # L3: separate 8-col denominator psum
                            pden = psS.tile([128, 8], F32, tag="psm")
                            for c in range(kt):
                                nc.tensor.matmul(pagg[:], oh_t[:, c, :], sc[:, c, 0:C],
                                                 start=(c == 0), stop=(c == kt - 1),
                                                 skip_group_check=True)
                            for c in range(kt):
                                nc.tensor.matmul(pden[:], oh_t[:, c, :], sc[:, c, C: C + 8],
                                                 start=(c == 0), stop=(c == kt - 1),
                                                 skip_group_check=True)
                            ptail = pden[:]
                        # self-loop term from own rows
                        se = wpool.tile([128, 8], BF16, tag="se")
                        nc.vector.tensor_tensor(se[:], hb[:, cfg["es"]: cfg["es"] + 8],
                                                hb[:, cfg["ed"]: cfg["ed"] + 8], OP.add)
                        s1 = wpool.tile([128, 8], BF16, tag="s1")
                        nc.scalar.activation(s1[:], se[:], AF.Exp)
                        s2 = wpool.tile([128, 8], BF16, tag="s2")
                        nc.scalar.activation(s2[:], se[:], AF.Exp, scale=NEG_SLOPE)
                        exs = wpool.tile([128, 8], BF16, tag="exs")
                        nc.vector.tensor_tensor(exs[:], s1[:], s2[:], OP.max)
                        t0 = wpool.tile([128, C], BF16, tag="t0g")
                        nc.vector.tensor_tensor(
                            t0[:].rearrange("p (j h) -> p j h", h=8),
                            hb[:, 0:C].rearrange("p (j h) -> p j h", h=8),
                            jh_bcast(exs[:], ch), OP.mult)
                        num = wpool.tile([128, C], F32, tag="num")
                        nc.vector.tensor_tensor(num[:], pagg[:, 0:C], t0[:], OP.add)
                        sden = wpool.tile([128, 8], F32, tag="sden")
                        nc.vector.tensor_tensor(sden[:], ptail, exs[:], OP.add)
                        rs = wpool.tile([128, 8], F32, tag="rs")
                        nc.vector.reciprocal(rs[:], sden[:])
                        agf = wpool.tile([128, C], F32, tag="agf")
                        nc.vector.tensor_tensor(
                            agf[:].rearrange("p (j h) -> p j h", h=8),
                            num[:].rearrange("p (j h) -> p j h", h=8),
                            jh_bcast(rs[:], ch), OP.mult)
                    epilogue(b, agf)
                    if l < 4:
                        if b == BA - 1:
                            allgather(l + 1, 0)
                        elif b == NB - 1:
                            allgather(l + 1, 1)

            # GCN prop is simpler; separate loop to keep code straight
            def prop_gcn(l, epilogue):
                cfg = LAYERS[l - 1]
                row, C = cfg["row"], cfg["C"]
                for b in range(NB):
                    klo, khi = int(K[b, 0]), int(K[b, 1])
                    kt = klo + khi
                    o = int(coff[b, 0])
                    G = gpool.tile([128, kt, row], BF16, tag="G")
                    nc.gpsimd.dma_gather(
                        out_ap=G[:, 0:klo, :], in_ap=TgA[l - 1][:],
                        idxs_ap=sidx_t[:, o * 8: o * 8 + (int(mx[b, 0]) + 15) // 16],
                        num_idxs=int(mx[b, 0]), num_idxs_reg=int(mx[b, 0]),
                        elem_size=row, single_packet=False)
                    nc.gpsimd.dma_gather(
                        out_ap=G[:, klo:kt, :], in_ap=TgB[l - 1][:],
                        idxs_ap=sidx_t[:, (o + klo) * 8: (o + klo) * 8 + (int(mx[b, 1]) + 15) // 16],
                        num_idxs=int(mx[b, 1]), num_idxs_reg=int(mx[b, 1]),
                        elem_size=row, single_packet=False)
                    hb = hbpool.tile([128, row], BF16, tag="hb")
                    if blk_rows[b] < 128:
                        nc.vector.memset(hb[:], 0.0)
                    nc.sync.dma_start(hb[0:blk_rows[b], :], tb_rows(l, b))
                    oh_t = ohpool.tile([128, kt, 128], BF16, tag="oh")
                    nc.sync.dma_start(oh_t[:], oh_in[:, o: o + kt, :])
                    pagg = psA.tile([128, C], F32, tag="pagg")
                    for c in range(kt):
                        nc.tensor.matmul(pagg[:], oh_t[:, c, :], G[:, c, 0:C],
                                         start=(c == 0), stop=(c == kt - 1),
                                         skip_group_check=True)
                    # self loop: sum += dis_d*h_d = row_d (rows are dis-scaled);
                    # the trailing *dis_d below completes the dis_d^2 norm
                    num = wpool.tile([128, C], F32, tag="num")
                    nc.vector.tensor_tensor(num[:], pagg[:], hb[:, 0:C], OP.add)
                    agf = wpool.tile([128, C], F32, tag="agf")
                    nc.vector.tensor_scalar(agf[:], num[:], dis_t[:, b: b + 1],
                                            None, OP.mult)
                    epilogue(b, agf)
                    if l < 4:
                        if b == BA - 1:
                            allgather(l + 1, 0)
                        elif b == NB - 1:
                            allgather(l + 1, 1)

            def layer_norm(t, Cn):
                st = wpool.tile([128, 6], F32, tag="st")
                nc.vector.bn_stats(st[:], t[:])
                mv = wpool.tile([128, 2], F32, tag="mv")
                nc.vector.bn_aggr(mv[:], st[:])
                sd = wpool.tile([128, 1], F32, tag="sd")
                nc.scalar.activation(sd[:], mv[:, 1:2], AF.Sqrt, bias=epscol[:])
                rstd = wpool.tile([128, 1], F32, tag="rstd")
                nc.vector.reciprocal(rstd[:], sd[:])
                xo = wpool.tile([128, Cn], BF16, tag="xo")
                nc.vector.tensor_scalar(xo[:], t[:], mv[:, 0:1], rstd[:],
                                        OP.subtract, OP.mult)
                return xo

            def transpose_to(x_ap, cols):
                pt = psE.tile([128, 128], BF16, tag="pse")
                nc.tensor.transpose(pt[0:cols, :], x_ap, id_t[:])
                xt_ = wpool.tile([cols, 128], BF16, tag=f"tr{cols}")
                nc.scalar.copy(xt_[:], pt[0:cols, :])
                return xt_

            # ---- L1 epilogue: LN -> x1, dense-2 (h2'|mlp2), T2 assembly ----
            def epi1(b, agf):
                t = wpool.tile([128, 256], F32, tag="t1")
                nc.vector.tensor_tensor(t[:], agf[:], mlp_sb[0][:, b, :], OP.add)
                x1 = layer_norm(t, 256)
                xta = transpose_to(x1[:, 0:128], 128)
                xtb = transpose_to(x1[:, 128:256], 128)
                ps2 = psA.tile([128, 256], F32, tag="pd")
                nc.tensor.matmul(ps2[:], xta[:], w2_t[:, 0, :], start=True, stop=False)
                nc.tensor.matmul(ps2[:], xtb[:], w2_t[:, 1, :], start=False, stop=True)
                as2 = apool.tile([128, 128], BF16, tag="as2")
                nc.scalar.activation(as2[:], ps2[:, 0:128], AF.Copy,
                                     scale=dis_t[:, b: b + 1])
                nc.sync.dma_start(tb_rows(2, b), as2[0:blk_rows[b], :])
                nc.vector.tensor_tensor(mlp_sb[1][:, b, :], ps2[:, 128:256], b2_t[:], OP.add)

            # ---- L2 epilogue: LN -> x2, dense-3, T3 assembly ----
            def epi2(b, agf):
                t = wpool.tile([128, 128], F32, tag="t2")
                nc.vector.tensor_tensor(t[:], agf[:], mlp_sb[1][:, b, :], OP.add)
                x2 = layer_norm(t, 128)
                xt2 = transpose_to(x2[:], 128)
                ps3a = psA.tile([128, 512], F32, tag="pd")
                nc.tensor.matmul(ps3a[:], xt2[:], w3_t[:, 0:512], start=True, stop=True)
                ps3b = psS.tile([128, 80], F32, tag="psm")
                nc.tensor.matmul(ps3b[:], xt2[:], w3_t[:, 512:592], start=True, stop=True)
                as3 = apool.tile([128, 640], BF16, tag="as3")
                nc.vector.memset(as3[:, 528:640], 0.0)
                nc.scalar.copy(as3[:, 0:512], ps3a[:])
                nc.vector.tensor_copy(as3[:, 512:528], ps3b[:, 64:80])
                nc.sync.dma_start(tb_rows(3, b), as3[0:blk_rows[b], :])
                nc.vector.tensor_tensor(mlp_sb[2][:, b, :], ps3b[:, 0:64], b3_t[:], OP.add)

            # ---- L3 epilogue: mean heads (inner 8), LN -> x3, T4 assembly ----
            def epi3(b, agf):
                mf = wpool.tile([128, 64], F32, tag="mf")
                nc.vector.tensor_reduce(
                    mf[:], agf[:].rearrange("p (j h) -> p j h", h=8),
                    mybir.AxisListType.X, OP.add)
                mh = wpool.tile([128, 64], F32, tag="mh")
                nc.vector.tensor_scalar(mh[:], mf[:], 0.125, None, OP.mult)
                t = wpool.tile([128, 64], F32, tag="t3")
                nc.vector.tensor_tensor(t[:], mh[:], mlp_sb[2][:, b, :], OP.add)
                x3 = layer_norm(t, 64)
                nc.vector.tensor_copy(x3_sb[:, b, :], x3[:])
                as4 = apool.tile([128, 128], BF16, tag="as4")
                nc.vector.memset(as4[:, 64:128], 0.0)
                nc.scalar.activation(as4[:, 0:64], x3[:], AF.Copy,
                                     scale=dis_t[:, b: b + 1])
                nc.sync.dma_start(tb_rows(4, b), as4[0:blk_rows[b], :])

            # ---- L4 epilogue: (agg @ W4) + (x3 @ m4_W) + bias ----
            def epi4(b, agf):
                a4 = wpool.tile([128, 64], BF16, tag="a4")
                nc.vector.tensor_copy(a4[:], agf[:])
                a4T = transpose_to(a4[:], 64)
                x3T = transpose_to(x3_sb[:, b, :], 64)
                ps4 = psS.tile([128, 2], F32, tag="psm")
                nc.tensor.matmul(ps4[:], a4T[:], w4_t[:, 0:2], start=True, stop=False)
                nc.tensor.matmul(ps4[:], x3T[:], w4_t[:, 2:4], start=False, stop=True)
                ot = wpool.tile([128, 2], F32, tag="ot")
                nc.vector.tensor_tensor(ot[:], ps4[:], b4_t[:], OP.add)
                nc.sync.dma_start(out_dram[b * 128: b * 128 + blk_rows[b], :], ot[0:blk_rows[b], :])

            prop(1, epi1)
            prop_gcn(2, epi2)
            prop(3, epi3)
            prop_gcn(4, epi4)

    nc.compile()
    return nc


def make_in_maps(inputs, sched, percore):
    NSH, NB = sched["NSH"], sched["NB"]
    wm = prep_weights(inputs)
    x = np.asarray(inputs["x"], np.float32)
    in_maps = []
    for c in range(P):
        xs = x[c * NSH: (c + 1) * NSH]
        pad = NB * 128 - NSH
        if pad:
            xs = np.concatenate([xs, np.zeros((pad, 1024), np.float32)], 0)
        xT = xs.astype(BF).reshape(NB, 128, 8, 128).transpose(0, 3, 2, 1).reshape(NB, 128, 1024).copy()
        pc = percore[c]
        in_maps.append(dict(
            xT=xT, w1=wm["w1"], w2=wm["w2"], w3=wm["w3"], w4=wm["w4"],
            bias1=wm["bias1"], bias2=wm["bias2"], bias3=wm["bias3"], bias4=wm["bias4"],
            ident=wm["ident"], dis=pc["dis"],
            sidx=pc["sidx"], oh=pc["oh"], ohT=pc["ohT"],
        ))
    return in_maps


def run(inputs, N=50000, trace=False):
    sched, percore = preprocess(np.asarray(inputs["edge_index"]), N)
    in_maps = make_in_maps(inputs, sched, percore)
    nc = build_nc(sched)
    res = run_bass_kernel_spmd(nc, in_maps, core_ids=list(range(P)), trace=trace)
    out = np.concatenate([res.results[c]["out"] for c in range(P)], axis=0)
    return out, res


def kernel(**inputs):
    out, _ = run(inputs, N=50000)
    return out.astype(np.float32)
